# revision 19
# baseline (speedup 1.0000x reference)
"""Baichuan attention (B=2, S=2048, H=4096, 32 heads x 128) on 8 TRN2 NeuronCores.

Tensor-parallel over heads (4 per core); o_proj row-parallel with the
partial-sum reduction done on host during unshard.

Per-core pipeline, all in bf16 matmuls (fp32 PSUM accumulate):
  Per batch b (sequential phases, all intermediates SBUF-resident):
    proj(b):  per 512-col s-chunk: Q/K/V projections; RoPE applied on the
              Q/K drains (ACT copy + partition-swap DMA + 3 bf16 DVE ops),
              results written to persistent bf16 SBUF tiles qT/kT [d,h,s]
              and v [k, kt, d].
    attn(b):  per 512-q chunk, per 128-k block, per head:
              scores MM (bf16, PSUM f32) -> causal mask on diagonal blocks
              via vector.tensor_mask_reduce -> Exp on ACT (bf16 out) ->
              ctx MM accumulate + den MM (one-hot lhsT -> per-head row of a
              shared [4,512] PSUM den tile; PSUM accumulates across k).
              Then one reciprocal_approx_fast per q-chunk, GpSimd
              partition-broadcast per head, fused normalize-to-bf16 drain.
    oproj(b): w_o bf16 streamed per 512-col out-chunk, ctx-stationary MMs,
              drains alternate Scalar/Vector, direct DMA to DRAM out.
Host: shards/transposes inputs, sums the 8 row-parallel partials.
"""
import os
import sys

for _p in ("/opt/trn_rl_repo", "/root/.axon_site/_ro/trn_rl_repo"):
    if os.path.isdir(_p) and _p not in sys.path:
        sys.path.insert(0, _p)

from contextlib import ExitStack

import ml_dtypes
import numpy as np

import concourse.bass as bass
import concourse.tile as tile
from concourse import bacc, mybir
from concourse.bass_utils import run_bass_kernel_spmd

F32 = mybir.dt.float32
BF16 = mybir.dt.bfloat16

B, S, H = 2, 2048, 4096
NH, HD = 32, 128
NCORES = 8
HPC = NH // NCORES          # heads per core = 4
DPC = HPC * HD              # dims per core = 512
ROPE_BASE = 10000.0

SBLK = 512                  # projection s-chunk
NSB = S // SBLK             # 4 s-chunks per batch
QC = 512                    # attention q-chunk
NQC = S // QC               # 4 q-chunks
NHT = H // 128              # 32 contraction tiles
NKT = S // 128              # 16 k-blocks per sequence
EXPF = mybir.ActivationFunctionType.Exp


def _build():
    nc = bacc.Bacc("TRN2", target_bir_lowering=False, debug=False,
                   num_devices=NCORES)

    xT = nc.dram_tensor("xT", [B, NHT, 128, S], BF16, kind="ExternalInput").ap()
    # wqkT[qk, dt, p, h*128+d] = w_{q|k}^T[128*h + p, 128*dt + d]
    wqkT = nc.dram_tensor("wqkT", [2, HPC, 128, NHT * 128], BF16,
                          kind="ExternalInput").ap()
    # wvT[p, h, d] = w_v^T[128*h + p, d]
    wvT = nc.dram_tensor("wvT", [128, NHT, DPC], BF16,
                         kind="ExternalInput").ap()
    # woT[p, oc, h, o] = w_o^T[128*h + p, 512*oc + o]
    woT = nc.dram_tensor("woT", [128, H // 512, HPC, 512], BF16,
                         kind="ExternalInput").ap()
    cosT = nc.dram_tensor("cosT", [HD, S], BF16, kind="ExternalInput").ap()
    sinTm = nc.dram_tensor("sinTm", [HD, S], BF16, kind="ExternalInput").ap()
    # iotas[:, i] = k + 128*i (mask_start per diag block), iotas[:, 4] = 512.0
    iotas = nc.dram_tensor("iotas", [128, 5], F32, kind="ExternalInput").ap()
    # id16[k, h, j] = 1 if j == h else 0, j<32  (den-matmul one-hot lhsT)
    id16 = nc.dram_tensor("id16", [128, HPC, 32], BF16,
                          kind="ExternalInput").ap()
    masks = nc.dram_tensor("masks", [128, 128], F32, kind="ExternalInput").ap()

    out = nc.dram_tensor("out", [B, S, H], F32, kind="ExternalOutput").ap()

    with tile.TileContext(nc) as tc, ExitStack() as top:
        persist = top.enter_context(tc.tile_pool(name="persist", bufs=1))

        cos_sb = persist.tile([HD, S], BF16)
        sin_sb = persist.tile([HD, S], BF16)
        iota_sb = persist.tile([128, 5], F32)
        id16_sb = persist.tile([128, HPC, 32], BF16)
        mask_sb = persist.tile([128, 128], F32)
        nc.sync.dma_start(out=cos_sb[:], in_=cosT[:])
        nc.sync.dma_start(out=sin_sb[:], in_=sinTm[:])
        nc.sync.dma_start(out=iota_sb[:], in_=iotas[:])
        nc.sync.dma_start(out=id16_sb[:], in_=id16[:])
        nc.sync.dma_start(out=mask_sb[:], in_=masks[:])

        wos_all = persist.tile([128, H // 512, HPC, 512], BF16, tag="wos")
        nc.sync.dma_start(out=wos_all[:], in_=woT[:])

        qT = persist.tile([128, HPC, S], BF16, tag="qT")
        kT = persist.tile([128, HPC, S], BF16, tag="kT")
        v_sb = persist.tile([128, NKT, DPC], BF16, tag="v")
        ctx_sb = persist.tile([128, HPC, S], BF16, tag="ctx")

        pend_po = []

        def pump_po(po_pool, oo_pool, n=1):
            for _ in range(min(n, len(pend_po))):
                b2, oc, st = pend_po.pop(0)
                po = po_pool.tile([128, 512], F32, tag="po", name="po")
                for h2 in range(HPC):
                    nc.tensor.matmul(
                        po[:],
                        ctx_sb[:, h2, st * 128:(st + 1) * 128],
                        wos_all[:, oc, h2, :],
                        start=(h2 == 0), stop=(h2 == HPC - 1))
                ot = oo_pool.tile([128, 512], F32, tag="ot")
                if st % 2 == 0:
                    nc.scalar.copy(ot[:], po[:])
                else:
                    nc.vector.tensor_copy(ot[:], po[:])
                nc.scalar.dma_start(
                    out=out[b2, st * 128:(st + 1) * 128,
                            oc * 512:(oc + 1) * 512],
                    in_=ot[:])

        for b in range(B):
            # ---------------- proj(b) ----------------
            with ExitStack() as ctx:
                xpool = ctx.enter_context(tc.tile_pool(name="xslab", bufs=34))
                wpool = ctx.enter_context(tc.tile_pool(name="wslab", bufs=6))
                rpool = ctx.enter_context(tc.tile_pool(name="rope", bufs=3))
                oop = ctx.enter_context(tc.tile_pool(name="pj_oo", bufs=4))
                pp = ctx.enter_context(tc.tile_pool(name="pj_psum", bufs=4,
                                                    space="PSUM"))
                pjpo = ctx.enter_context(tc.tile_pool(name="pj_po", bufs=2,
                                                      space="PSUM"))

                for sb in range(NSB):
                    s0 = sb * SBLK
                    xsl = []
                    for h in range(NHT):
                        xs = xpool.tile([128, SBLK], BF16, tag="xs")
                        nc.sync.dma_start(out=xs[:],
                                          in_=xT[b, h, :, s0:s0 + SBLK])
                        xsl.append(xs)

                    # Q and K passes: out [d(head dt), s] with rope on drain
                    for qk in range(2):
                        for dt in range(HPC):
                            ps = pp.tile([128, SBLK], F32, tag="pp",
                                         name=f"pj{qk}{dt}")
                            w = wpool.tile([128, NHT, 128], BF16, tag="w", bufs=3)
                            nc.sync.dma_start(out=w[:], in_=wqkT[qk, dt])
                            for h in range(NHT):
                                nc.tensor.matmul(
                                    ps[:], w[:, h, :], xsl[h][:],
                                    start=(h == 0), stop=(h == NHT - 1))
                            # rope drain -> (qT|kT)[:, dt, s0:s0+SBLK]
                            dst = (qT if qk == 0 else kT)[:, dt, s0:s0 + SBLK]
                            qsb = rpool.tile([128, SBLK], BF16, tag="qsb")
                            nc.scalar.copy(qsb[:], ps[:])
                            qsw = rpool.tile([128, SBLK], BF16, tag="qsw")
                            nc.scalar.dma_start(out=qsw[0:64, :],
                                                in_=qsb[64:128, :])
                            nc.scalar.dma_start(out=qsw[64:128, :],
                                                in_=qsb[0:64, :])
                            t1 = rpool.tile([128, SBLK], BF16, tag="t1")
                            nc.vector.tensor_mul(t1[:], qsb[:],
                                                 cos_sb[:, s0:s0 + SBLK])
                            t2 = rpool.tile([128, SBLK], BF16, tag="t2")
                            nc.vector.tensor_mul(t2[:], qsw[:],
                                                 sin_sb[:, s0:s0 + SBLK])
                            nc.vector.tensor_add(dst, t1[:], t2[:])
                            pump_po(pjpo, oop)

                    # V pass: out [s-tile, d] tile-major
                    wvg = []
                    for g in range(4):
                        wv = wpool.tile([128, 8, DPC], BF16, tag="wv", bufs=3)
                        nc.sync.dma_start(out=wv[:],
                                          in_=wvT[:, 8 * g:8 * g + 8, :])
                        wvg.append(wv)
                    for st in range(SBLK // 128):
                        psv = pp.tile([128, DPC], F32, tag="pp",
                                      name=f"pjv{st}")
                        for h in range(NHT):
                            nc.tensor.matmul(
                                psv[:],
                                xsl[h][:, st * 128:(st + 1) * 128],
                                wvg[h // 8][:, h % 8, :],
                                start=(h == 0), stop=(h == NHT - 1))
                        nc.vector.tensor_copy(
                            v_sb[:, (s0 + st * 128) // 128, :], psv[:])
                        pump_po(pjpo, oop)

            # ---------------- attn(b) ----------------
            with ExitStack() as ctx:
                prpool = ctx.enter_context(tc.tile_pool(name="at_pr", bufs=8))
                smpool = ctx.enter_context(tc.tile_pool(name="at_sm", bufs=6))
                ps_s = ctx.enter_context(tc.tile_pool(name="ps_s", bufs=2,
                                                      space="PSUM"))
                atpo = ctx.enter_context(tc.tile_pool(name="at_po", bufs=1,
                                                      space="PSUM"))
                oop2 = ctx.enter_context(tc.tile_pool(name="at_oo", bufs=4))
                ps_c = ctx.enter_context(tc.tile_pool(name="ps_c", bufs=4,
                                                      space="PSUM"))
                ps_d = ctx.enter_context(tc.tile_pool(name="ps_d", bufs=1,
                                                      space="PSUM"))

                for qc in range(NQC):
                    q0 = qc * QC
                    nkt = 4 * qc + 4
                    pden = ps_d.tile([32, QC], F32, tag="pden", name="pden")
                    pc = []
                    # one pass per head (PSUM: 4 pc + 3 pss + 1 den).
                    # Software-pipelined 2 deep: ctx/den MMs of block kt
                    # issue after the score MM of block kt+2, hiding the
                    # mask+exp latency.  Diagonal blocks are narrowed to
                    # their unmasked q-columns (memset zeros the rest of
                    # prt so ctx/den stay full-width).
                    for h in range(HPC):
                        pch = ps_c.tile([128, QC], F32, tag="pc",
                                        name=f"pc{h}")
                        pc.append(pch)
                        pending = []

                        def flush(h=h, pch=pch):
                            kt, prt = pending.pop(0)
                            nc.tensor.matmul(
                                pch[:],
                                v_sb[:, kt, h * HD:(h + 1) * HD],
                                prt[:],
                                start=(kt == 0), stop=(kt == nkt - 1))
                            nc.tensor.matmul(
                                pden[0:32, :],
                                id16_sb[:, h, :],
                                prt[:],
                                start=(h == 0 and kt == 0),
                                stop=(h == HPC - 1 and kt == nkt - 1))

                        for kt in range(nkt):
                            i = kt - 4 * qc
                            c0 = 128 * i if i >= 0 else 0
                            pss = ps_s.tile([128, QC], F32, tag="pss",
                                            name="pss")
                            nc.tensor.matmul(
                                pss[:, c0:QC],
                                kT[:, h, kt * 128:(kt + 1) * 128],
                                qT[:, h, q0 + c0:q0 + QC],
                                start=True, stop=True)
                            if len(pending) == 2:
                                flush()
                            if i >= 0:
                                nc.vector.tensor_add(
                                    pss[:, c0:c0 + 128], pss[:, c0:c0 + 128],
                                    mask_sb[:])
                            pr = prpool.tile([128, QC], BF16, tag="pr",
                                             name="pr")
                            nc.scalar.activation(out=pr[:, c0:QC],
                                                 in_=pss[:, c0:QC],
                                                 func=EXPF)
                            if c0 > 0:
                                nc.vector.memset(pr[:, 0:c0], 0)
                            pending.append((kt, pr))
                            pump_po(atpo, oop2)
                        while pending:
                            flush()
                    rec = smpool.tile([HPC, QC], F32, tag="rec")
                    nc.vector.reciprocal(out=rec[:], in_=pden[0:HPC, :])
                    for h in range(HPC):
                        rh = smpool.tile([1, QC], F32, tag=f"rh{h}", bufs=2)
                        nc.scalar.dma_start(out=rh[:], in_=rec[h:h + 1, :])
                        rbc = smpool.tile([128, QC], F32, tag="rbc")
                        nc.gpsimd.partition_broadcast(rbc[:], rh[:])
                        nc.vector.tensor_mul(ctx_sb[:, h, q0:q0 + QC],
                                             pc[h][:], rbc[:])
                    for oc in range(H // 512):
                        for st in range(4 * qc, 4 * qc + 4):
                            pend_po.append((b, oc, st))

            if b == B - 1:
                with ExitStack() as ctx:
                    oop3 = ctx.enter_context(tc.tile_pool(name="oo3", bufs=4))
                    po3 = ctx.enter_context(tc.tile_pool(name="po3", bufs=4,
                                                         space="PSUM"))
                    pump_po(po3, oop3, n=len(pend_po))

    nc.compile()
    return nc


_CACHE = {}


def _host_prep(x, w_pack, w_o):
    """Build per-core input maps (sharding + layout prep)."""
    x = np.asarray(x, dtype=np.float32)
    w_pack = np.asarray(w_pack, dtype=np.float32)
    w_o = np.asarray(w_o, dtype=np.float32)

    xT = np.ascontiguousarray(
        x.transpose(0, 2, 1).reshape(B, NHT, 128, S)
        .astype(ml_dtypes.bfloat16))                     # [B, 32, 128, S]

    inv_freq = 1.0 / (ROPE_BASE ** (np.arange(0, HD, 2, dtype=np.float32) / HD))
    t = np.arange(S, dtype=np.float32)
    freqs = np.outer(t, inv_freq)                            # [S, HD/2]
    emb = np.concatenate([freqs, freqs], axis=-1)            # [S, HD]
    cosT = np.ascontiguousarray(
        np.cos(emb).T.astype(ml_dtypes.bfloat16))            # [HD, S]
    sinT = np.sin(emb).T.astype(np.float32)
    sinTm = np.concatenate([-sinT[:HD // 2], sinT[HD // 2:]], axis=0)
    sinTm = np.ascontiguousarray(sinTm.astype(ml_dtypes.bfloat16))

    kk = np.arange(128, dtype=np.float32)
    iotas = np.stack([kk + 128 * i for i in range(4)]
                     + [np.full(128, 512.0, np.float32)], axis=1)
    iotas = np.ascontiguousarray(iotas)                      # [128, 5]

    kk2 = np.arange(128)[:, None]
    qq = np.arange(128)[None, :]
    masks = np.ascontiguousarray(
        np.where(kk2 <= qq, 0.0, -1.0e30).astype(np.float32))  # [128, 128]

    id16 = np.zeros((128, HPC, 32), dtype=np.float32)
    for h in range(HPC):
        id16[:, h, h] = 1.0
    id16 = np.ascontiguousarray(id16.astype(ml_dtypes.bfloat16))

    scale = float(HD) ** -0.5
    in_maps = []
    for c in range(NCORES):
        r0 = c * DPC
        wq = w_pack[r0:r0 + DPC, :] * scale                  # [512, H]
        wk = w_pack[H + r0:H + r0 + DPC, :]
        wv = w_pack[2 * H + r0:2 * H + r0 + DPC, :]
        # wqkT[qk, dt, p, 128h+d] = w^T[128h+p, 128dt+d]
        wqkT = np.stack([wq.T, wk.T], axis=0)                # [2, H, 512]
        wqkT = wqkT.reshape(2, NHT, 128, HPC, 128)           # [2,h,p,dt,d]
        wqkT = wqkT.transpose(0, 3, 2, 1, 4).reshape(2, HPC, 128, NHT * 128)
        wqkT = np.ascontiguousarray(wqkT.astype(ml_dtypes.bfloat16))
        # wvT[p, h, d] = w_v^T[128h+p, d]
        wvT = wv.T.reshape(NHT, 128, DPC).transpose(1, 0, 2)
        wvT = np.ascontiguousarray(wvT.astype(ml_dtypes.bfloat16))
        # woT[p, oc, h, o] = w_o^T[128h+p, 512oc+o]
        woT = w_o[:, r0:r0 + DPC].T.reshape(HPC, 128, H // 512, 512)
        woT = woT.transpose(1, 2, 0, 3)
        woT = np.ascontiguousarray(woT.astype(ml_dtypes.bfloat16))
        in_maps.append({
            "xT": xT, "wqkT": wqkT, "wvT": wvT, "woT": woT,
            "cosT": cosT, "sinTm": sinTm, "iotas": iotas, "id16": id16,
            "masks": masks,
        })
    return in_maps


def kernel(x, w_pack, w_o, _trace=False, _trace_kwargs=None):
    if "nc" not in _CACHE:
        _CACHE["nc"] = _build()
    nc = _CACHE["nc"]

    in_maps = _host_prep(x, w_pack, w_o)
    res = run_bass_kernel_spmd(nc, in_maps, list(range(NCORES)),
                               trace=_trace, **(_trace_kwargs or {}))
    acc = res.results[0]["out"].astype(np.float32)
    for c in range(1, NCORES):
        acc = acc + res.results[c]["out"]
    if _trace:
        kernel.last_results = res
    return acc


# revision 20
# speedup vs baseline: 1.0218x; 1.0218x over previous
"""Baichuan attention (B=2, S=2048, H=4096, 32 heads x 128) on 8 TRN2 NeuronCores.

Tensor-parallel over heads (4 per core); o_proj row-parallel with the
partial-sum reduction done on host during unshard.

Per-core pipeline, all in bf16 matmuls (fp32 PSUM accumulate):
  Per batch b (sequential phases, all intermediates SBUF-resident):
    proj(b):  per 512-col s-chunk: Q/K/V projections; RoPE applied on the
              Q/K drains (ACT copy + partition-swap DMA + 3 bf16 DVE ops),
              results written to persistent bf16 SBUF tiles qT/kT [d,h,s]
              and v [k, kt, d].
    attn(b):  per 512-q chunk, per 128-k block, per head:
              scores MM (bf16, PSUM f32) -> causal mask on diagonal blocks
              via vector.tensor_mask_reduce -> Exp on ACT (bf16 out) ->
              ctx MM accumulate + den MM (one-hot lhsT -> per-head row of a
              shared [4,512] PSUM den tile; PSUM accumulates across k).
              Then one reciprocal_approx_fast per q-chunk, GpSimd
              partition-broadcast per head, fused normalize-to-bf16 drain.
    oproj(b): w_o bf16 streamed per 512-col out-chunk, ctx-stationary MMs,
              drains alternate Scalar/Vector, direct DMA to DRAM out.
Host: shards/transposes inputs, sums the 8 row-parallel partials.
"""
import os
import sys

for _p in ("/opt/trn_rl_repo", "/root/.axon_site/_ro/trn_rl_repo"):
    if os.path.isdir(_p) and _p not in sys.path:
        sys.path.insert(0, _p)

from contextlib import ExitStack

import ml_dtypes
import numpy as np

import concourse.bass as bass
import concourse.tile as tile
from concourse import bacc, mybir
from concourse.bass_utils import run_bass_kernel_spmd

F32 = mybir.dt.float32
BF16 = mybir.dt.bfloat16

B, S, H = 2, 2048, 4096
NH, HD = 32, 128
NCORES = 8
HPC = NH // NCORES          # heads per core = 4
DPC = HPC * HD              # dims per core = 512
ROPE_BASE = 10000.0

SBLK = 512                  # projection s-chunk
NSB = S // SBLK             # 4 s-chunks per batch
QC = 512                    # attention q-chunk
NQC = S // QC               # 4 q-chunks
NHT = H // 128              # 32 contraction tiles
NKT = S // 128              # 16 k-blocks per sequence
EXPF = mybir.ActivationFunctionType.Exp


def _build():
    nc = bacc.Bacc("TRN2", target_bir_lowering=False, debug=False,
                   num_devices=NCORES)

    xT = nc.dram_tensor("xT", [B, NHT, 128, S], BF16, kind="ExternalInput").ap()
    # wqkT[qk, dt, p, h*128+d] = w_{q|k}^T[128*h + p, 128*dt + d]
    wqkT = nc.dram_tensor("wqkT", [2, HPC, 128, NHT * 128], BF16,
                          kind="ExternalInput").ap()
    # wvT[p, h, d] = w_v^T[128*h + p, d]
    wvT = nc.dram_tensor("wvT", [128, NHT, DPC], BF16,
                         kind="ExternalInput").ap()
    # woT[p, oc, h, o] = w_o^T[128*h + p, 512*oc + o]
    woT = nc.dram_tensor("woT", [128, H // 512, HPC, 512], BF16,
                         kind="ExternalInput").ap()
    cosT = nc.dram_tensor("cosT", [HD, S], BF16, kind="ExternalInput").ap()
    sinTm = nc.dram_tensor("sinTm", [HD, S], BF16, kind="ExternalInput").ap()
    # iotas[:, i] = k + 128*i (mask_start per diag block), iotas[:, 4] = 512.0
    iotas = nc.dram_tensor("iotas", [128, 5], F32, kind="ExternalInput").ap()
    # id16[k, h, j] = 1 if j == h else 0, j<128  (den-matmul one-hot lhsT;
    # full 128-col stationary so the den MM runs at full-M speed)
    id16 = nc.dram_tensor("id16", [128, HPC, 128], BF16,
                          kind="ExternalInput").ap()
    masks = nc.dram_tensor("masks", [128, 128], F32, kind="ExternalInput").ap()

    out = nc.dram_tensor("out", [B, S, H], F32, kind="ExternalOutput").ap()

    with tile.TileContext(nc) as tc, ExitStack() as top:
        persist = top.enter_context(tc.tile_pool(name="persist", bufs=1))

        cos_sb = persist.tile([HD, S], BF16)
        sin_sb = persist.tile([HD, S], BF16)
        iota_sb = persist.tile([128, 5], F32)
        id16_sb = persist.tile([128, HPC, 128], BF16)
        mask_sb = persist.tile([128, 128], F32)
        nc.sync.dma_start(out=cos_sb[:], in_=cosT[:])
        nc.sync.dma_start(out=sin_sb[:], in_=sinTm[:])
        nc.sync.dma_start(out=iota_sb[:], in_=iotas[:])
        nc.sync.dma_start(out=id16_sb[:], in_=id16[:])
        nc.sync.dma_start(out=mask_sb[:], in_=masks[:])

        wos_all = persist.tile([128, H // 512, HPC, 512], BF16, tag="wos")

        qT = persist.tile([128, HPC, S], BF16, tag="qT")
        kT = persist.tile([128, HPC, S], BF16, tag="kT")
        v_sb = persist.tile([128, NKT, DPC], BF16, tag="v")
        ctx_sb = persist.tile([128, HPC, S], BF16, tag="ctx")

        pend_po = []

        def pump_po(po_pool, oo_pool, n=1):
            for _ in range(min(n, len(pend_po))):
                b2, oc, st = pend_po.pop(0)
                po = po_pool.tile([128, 512], F32, tag="po", name="po")
                for h2 in range(HPC):
                    nc.tensor.matmul(
                        po[:],
                        ctx_sb[:, h2, st * 128:(st + 1) * 128],
                        wos_all[:, oc, h2, :],
                        start=(h2 == 0), stop=(h2 == HPC - 1))
                ot = oo_pool.tile([128, 512], F32, tag="ot")
                if st % 2 == 0:
                    nc.scalar.copy(ot[:], po[:])
                else:
                    nc.vector.tensor_copy(ot[:], po[:])
                nc.scalar.dma_start(
                    out=out[b2, st * 128:(st + 1) * 128,
                            oc * 512:(oc + 1) * 512],
                    in_=ot[:])

        for b in range(B):
            # ---------------- proj(b) ----------------
            with ExitStack() as ctx:
                xpool = ctx.enter_context(tc.tile_pool(name="xslab", bufs=34))
                wpool = ctx.enter_context(tc.tile_pool(name="wslab", bufs=6))
                rpool = ctx.enter_context(tc.tile_pool(name="rope", bufs=3))
                oop = ctx.enter_context(tc.tile_pool(name="pj_oo", bufs=4))
                pp = ctx.enter_context(tc.tile_pool(name="pj_psum", bufs=4,
                                                    space="PSUM"))
                pjpo = ctx.enter_context(tc.tile_pool(name="pj_po", bufs=2,
                                                      space="PSUM"))

                for sb in range(NSB):
                    s0 = sb * SBLK
                    xsl = []
                    for h in range(NHT):
                        xs = xpool.tile([128, SBLK], BF16, tag="xs")
                        nc.sync.dma_start(out=xs[:],
                                          in_=xT[b, h, :, s0:s0 + SBLK])
                        xsl.append(xs)
                    if b == 0 and sb == 0:
                        nc.scalar.dma_start(out=wos_all[:], in_=woT[:])

                    # Q and K passes: out [d(head dt), s] with rope on drain
                    for qk in range(2):
                        for dt in range(HPC):
                            ps = pp.tile([128, SBLK], F32, tag="pp",
                                         name=f"pj{qk}{dt}")
                            w = wpool.tile([128, NHT, 128], BF16, tag="w", bufs=3)
                            nc.sync.dma_start(out=w[:], in_=wqkT[qk, dt])
                            for h in range(NHT):
                                nc.tensor.matmul(
                                    ps[:], w[:, h, :], xsl[h][:],
                                    start=(h == 0), stop=(h == NHT - 1))
                            # rope drain -> (qT|kT)[:, dt, s0:s0+SBLK]
                            dst = (qT if qk == 0 else kT)[:, dt, s0:s0 + SBLK]
                            qsb = rpool.tile([128, SBLK], BF16, tag="qsb")
                            nc.scalar.copy(qsb[:], ps[:])
                            qsw = rpool.tile([128, SBLK], BF16, tag="qsw")
                            nc.scalar.dma_start(out=qsw[0:64, :],
                                                in_=qsb[64:128, :])
                            nc.scalar.dma_start(out=qsw[64:128, :],
                                                in_=qsb[0:64, :])
                            t1 = rpool.tile([128, SBLK], BF16, tag="t1")
                            nc.vector.tensor_mul(t1[:], qsb[:],
                                                 cos_sb[:, s0:s0 + SBLK])
                            t2 = rpool.tile([128, SBLK], BF16, tag="t2")
                            nc.vector.tensor_mul(t2[:], qsw[:],
                                                 sin_sb[:, s0:s0 + SBLK])
                            nc.vector.tensor_add(dst, t1[:], t2[:])
                            pump_po(pjpo, oop)

                    # V pass: out [s-tile, d] tile-major
                    wvg = []
                    for g in range(4):
                        wv = wpool.tile([128, 8, DPC], BF16, tag="wv", bufs=3)
                        nc.sync.dma_start(out=wv[:],
                                          in_=wvT[:, 8 * g:8 * g + 8, :])
                        wvg.append(wv)
                    for st in range(SBLK // 128):
                        psv = pp.tile([128, DPC], F32, tag="pp",
                                      name=f"pjv{st}")
                        for h in range(NHT):
                            nc.tensor.matmul(
                                psv[:],
                                xsl[h][:, st * 128:(st + 1) * 128],
                                wvg[h // 8][:, h % 8, :],
                                start=(h == 0), stop=(h == NHT - 1))
                        nc.vector.tensor_copy(
                            v_sb[:, (s0 + st * 128) // 128, :], psv[:])
                        pump_po(pjpo, oop)

            # ---------------- attn(b) ----------------
            with ExitStack() as ctx:
                prpool = ctx.enter_context(tc.tile_pool(name="at_pr", bufs=8))
                smpool = ctx.enter_context(tc.tile_pool(name="at_sm", bufs=6))
                ps_s = ctx.enter_context(tc.tile_pool(name="ps_s", bufs=2,
                                                      space="PSUM"))
                atpo = ctx.enter_context(tc.tile_pool(name="at_po", bufs=1,
                                                      space="PSUM"))
                oop2 = ctx.enter_context(tc.tile_pool(name="at_oo", bufs=4))
                ps_c = ctx.enter_context(tc.tile_pool(name="ps_c", bufs=4,
                                                      space="PSUM"))
                ps_d = ctx.enter_context(tc.tile_pool(name="ps_d", bufs=1,
                                                      space="PSUM"))

                for qc in range(NQC):
                    q0 = qc * QC
                    nkt = 4 * qc + 4
                    pden = ps_d.tile([128, QC], F32, tag="pden", name="pden")
                    pc = []
                    # one pass per head (PSUM: 4 pc + 3 pss + 1 den).
                    # Software-pipelined 2 deep: ctx/den MMs of block kt
                    # issue after the score MM of block kt+2, hiding the
                    # mask+exp latency.  Diagonal blocks are narrowed to
                    # their unmasked q-columns (memset zeros the rest of
                    # prt so ctx/den stay full-width).
                    for h in range(HPC):
                        pch = ps_c.tile([128, QC], F32, tag="pc",
                                        name=f"pc{h}")
                        pc.append(pch)
                        pending = []

                        def flush(h=h, pch=pch):
                            kt, prt = pending.pop(0)
                            nc.tensor.matmul(
                                pch[:],
                                v_sb[:, kt, h * HD:(h + 1) * HD],
                                prt[:],
                                start=(kt == 0), stop=(kt == nkt - 1))
                            nc.tensor.matmul(
                                pden[0:128, :],
                                id16_sb[:, h, :],
                                prt[:],
                                start=(h == 0 and kt == 0),
                                stop=(h == HPC - 1 and kt == nkt - 1))

                        for kt in range(nkt):
                            i = kt - 4 * qc
                            c0 = 128 * i if i >= 0 else 0
                            pss = ps_s.tile([128, QC], F32, tag="pss",
                                            name="pss")
                            nc.tensor.matmul(
                                pss[:, c0:QC],
                                kT[:, h, kt * 128:(kt + 1) * 128],
                                qT[:, h, q0 + c0:q0 + QC],
                                start=True, stop=True)
                            if len(pending) == 2:
                                flush()
                            if i >= 0:
                                nc.vector.tensor_add(
                                    pss[:, c0:c0 + 128], pss[:, c0:c0 + 128],
                                    mask_sb[:])
                            pr = prpool.tile([128, QC], BF16, tag="pr",
                                             name="pr")
                            nc.scalar.activation(out=pr[:, c0:QC],
                                                 in_=pss[:, c0:QC],
                                                 func=EXPF)
                            if c0 > 0:
                                nc.vector.memset(pr[:, 0:c0], 0)
                            pending.append((kt, pr))
                            pump_po(atpo, oop2)
                        while pending:
                            flush()
                    rec = smpool.tile([HPC, QC], F32, tag="rec")
                    nc.vector.reciprocal(out=rec[:], in_=pden[0:HPC, :])
                    for h in range(HPC):
                        rh = smpool.tile([1, QC], F32, tag=f"rh{h}", bufs=2)
                        nc.scalar.dma_start(out=rh[:], in_=rec[h:h + 1, :])
                        rbc = smpool.tile([128, QC], F32, tag="rbc")
                        nc.gpsimd.partition_broadcast(rbc[:], rh[:])
                        nc.vector.tensor_mul(ctx_sb[:, h, q0:q0 + QC],
                                             pc[h][:], rbc[:])
                    for oc in range(H // 512):
                        for st in range(4 * qc, 4 * qc + 4):
                            pend_po.append((b, oc, st))

            if b == B - 1:
                with ExitStack() as ctx:
                    oop3 = ctx.enter_context(tc.tile_pool(name="oo3", bufs=4))
                    po3 = ctx.enter_context(tc.tile_pool(name="po3", bufs=4,
                                                         space="PSUM"))
                    pump_po(po3, oop3, n=len(pend_po))

    nc.compile()
    return nc


_CACHE = {}


def _host_prep(x, w_pack, w_o):
    """Build per-core input maps (sharding + layout prep)."""
    x = np.asarray(x, dtype=np.float32)
    w_pack = np.asarray(w_pack, dtype=np.float32)
    w_o = np.asarray(w_o, dtype=np.float32)

    xT = np.ascontiguousarray(
        x.transpose(0, 2, 1).reshape(B, NHT, 128, S)
        .astype(ml_dtypes.bfloat16))                     # [B, 32, 128, S]

    inv_freq = 1.0 / (ROPE_BASE ** (np.arange(0, HD, 2, dtype=np.float32) / HD))
    t = np.arange(S, dtype=np.float32)
    freqs = np.outer(t, inv_freq)                            # [S, HD/2]
    emb = np.concatenate([freqs, freqs], axis=-1)            # [S, HD]
    cosT = np.ascontiguousarray(
        np.cos(emb).T.astype(ml_dtypes.bfloat16))            # [HD, S]
    sinT = np.sin(emb).T.astype(np.float32)
    sinTm = np.concatenate([-sinT[:HD // 2], sinT[HD // 2:]], axis=0)
    sinTm = np.ascontiguousarray(sinTm.astype(ml_dtypes.bfloat16))

    kk = np.arange(128, dtype=np.float32)
    iotas = np.stack([kk + 128 * i for i in range(4)]
                     + [np.full(128, 512.0, np.float32)], axis=1)
    iotas = np.ascontiguousarray(iotas)                      # [128, 5]

    kk2 = np.arange(128)[:, None]
    qq = np.arange(128)[None, :]
    masks = np.ascontiguousarray(
        np.where(kk2 <= qq, 0.0, -1.0e30).astype(np.float32))  # [128, 128]

    id16 = np.zeros((128, HPC, 128), dtype=np.float32)
    for h in range(HPC):
        id16[:, h, h] = 1.0
    id16 = np.ascontiguousarray(id16.astype(ml_dtypes.bfloat16))

    scale = float(HD) ** -0.5
    in_maps = []
    for c in range(NCORES):
        r0 = c * DPC
        wq = w_pack[r0:r0 + DPC, :] * scale                  # [512, H]
        wk = w_pack[H + r0:H + r0 + DPC, :]
        wv = w_pack[2 * H + r0:2 * H + r0 + DPC, :]
        # wqkT[qk, dt, p, 128h+d] = w^T[128h+p, 128dt+d]
        wqkT = np.stack([wq.T, wk.T], axis=0)                # [2, H, 512]
        wqkT = wqkT.reshape(2, NHT, 128, HPC, 128)           # [2,h,p,dt,d]
        wqkT = wqkT.transpose(0, 3, 2, 1, 4).reshape(2, HPC, 128, NHT * 128)
        wqkT = np.ascontiguousarray(wqkT.astype(ml_dtypes.bfloat16))
        # wvT[p, h, d] = w_v^T[128h+p, d]
        wvT = wv.T.reshape(NHT, 128, DPC).transpose(1, 0, 2)
        wvT = np.ascontiguousarray(wvT.astype(ml_dtypes.bfloat16))
        # woT[p, oc, h, o] = w_o^T[128h+p, 512oc+o]
        woT = w_o[:, r0:r0 + DPC].T.reshape(HPC, 128, H // 512, 512)
        woT = woT.transpose(1, 2, 0, 3)
        woT = np.ascontiguousarray(woT.astype(ml_dtypes.bfloat16))
        in_maps.append({
            "xT": xT, "wqkT": wqkT, "wvT": wvT, "woT": woT,
            "cosT": cosT, "sinTm": sinTm, "iotas": iotas, "id16": id16,
            "masks": masks,
        })
    return in_maps


def kernel(x, w_pack, w_o, _trace=False, _trace_kwargs=None):
    if "nc" not in _CACHE:
        _CACHE["nc"] = _build()
    nc = _CACHE["nc"]

    in_maps = _host_prep(x, w_pack, w_o)
    res = run_bass_kernel_spmd(nc, in_maps, list(range(NCORES)),
                               trace=_trace, **(_trace_kwargs or {}))
    acc = res.results[0]["out"].astype(np.float32)
    for c in range(1, NCORES):
        acc = acc + res.results[c]["out"]
    if _trace:
        kernel.last_results = res
    return acc
